# revision 2
# baseline (speedup 1.0000x reference)
"""Scatter-add of active-site feature rows into a dense (B, L, C) output,
distributed over 8 NeuronCores (data-parallel over the batch axis).

Core m owns flat output rows [m*8192, (m+1)*8192), split into 64 blocks of
128 consecutive rows. Rows are bucketed by (core, block); within a block the
distinct target positions are compacted to ranks 0..O-1. Each block is a
one-hot matmul

    acc[q, c] = sum_k 1{rank[k] == q} * feats[k, c]

in fp32 PSUM (duplicates sum exactly), after which only the O occupied rows
are scatter-stored to DRAM via indirect DMA. Untouched output rows are never
written: run_bass_via_pjrt donates zero-initialized output buffers (and the
native path pre-zeros them), so they hold exact zeros.

SPMD uniformity: each core orders its own 64 blocks by occupancy; slot s's
row capacity KS[s] / store capacity QS[s] are maxima across the 8 cores, so
one program serves all cores while per-core data (features, ranks, store
indices) fills the slots. Store-index padding points at a known-empty row of
that core (writing zeros there is a no-op by construction).

Features move as bf16 (the matmul's PSUM accumulation stays fp32), halving
load bytes for ~3e-3 relative output error.
"""

import os

import numpy as np
import ml_dtypes

import concourse.bacc as bacc
import concourse.bass as bass
import concourse.mybir as mybir
import concourse.tile as tile
from concourse.bass_utils import run_bass_kernel_spmd

N_CORES = 8
B = 16
L = 4096
C = 512
POS_PER_CORE = B * L // N_CORES  # 8192
NBLK = 64  # 128-row blocks per core

SORT = os.environ.get("K_SORT", "O")  # block ordering for slot capacities
ROUND = int(os.environ.get("K_ROUND", "1"))  # round KS up to this multiple
FDT = os.environ.get("K_FDT", "bf16")  # feature dtype on the wire
FBUFS = int(os.environ.get("K_FBUFS", "12"))
OBUFS = int(os.environ.get("K_OBUFS", "10"))
MBUFS = int(os.environ.get("K_MBUFS", "6"))
PSBUFS = int(os.environ.get("K_PSBUFS", "8"))
COPY_ENG = os.environ.get("K_COPY", "mix")  # dve | mix

_PROGRAM_CACHE: dict = {}


def _build_program(KS: tuple, QS: tuple):
    f32 = mybir.dt.float32
    fdt = mybir.dt.bfloat16 if FDT == "bf16" else f32
    i32 = mybir.dt.int32
    CUM = np.concatenate([[0], np.cumsum(KS)]).astype(np.int64)
    TOTK = int(CUM[-1])

    nc = bacc.Bacc(
        "TRN2",
        target_bir_lowering=False,
        debug=False,
        enable_asserts=False,
        num_devices=N_CORES,
    )
    feats_d = nc.dram_tensor("feats", [TOTK, C], fdt, kind="ExternalInput")
    rank_d = nc.dram_tensor("rank", [128, NBLK], f32, kind="ExternalInput")
    sidx_d = nc.dram_tensor("sidx", [128, NBLK], i32, kind="ExternalInput")
    iota_d = nc.dram_tensor("iota", [128, 128], f32, kind="ExternalInput")
    out_d = nc.dram_tensor("out", [POS_PER_CORE, C], f32, kind="ExternalOutput")

    eq = mybir.AluOpType.is_equal

    with tile.TileContext(nc) as tc:
        with (
            tc.tile_pool(name="const", bufs=1) as constp,
            tc.tile_pool(name="fpool", bufs=FBUFS) as fpool,
            tc.tile_pool(name="opool", bufs=OBUFS) as opool,
            tc.tile_pool(name="mpool", bufs=MBUFS) as mpool,
            tc.tile_pool(name="psum", bufs=PSBUFS, space="PSUM") as pspool,
        ):
            iota_t = constp.tile([128, 128], f32)
            nc.scalar.dma_start(iota_t[:], iota_d.ap())
            rank_t = constp.tile([128, NBLK], f32)
            nc.scalar.dma_start(rank_t[:], rank_d.ap())
            sidx_t = constp.tile([128, NBLK], i32)
            nc.scalar.dma_start(sidx_t[:], sidx_d.ap())

            for s in range(NBLK):
                K = int(KS[s])
                Q = int(QS[s])
                if K == 0 or Q == 0:
                    continue
                ft = fpool.tile([K, C], fdt, tag="ft")
                nc.sync.dma_start(ft[:], feats_d.ap()[int(CUM[s]) : int(CUM[s]) + K, :])
                m = mpool.tile([K, 128], fdt, tag="m")
                nc.vector.tensor_scalar(
                    m[:], iota_t[:K, :], rank_t[:K, s : s + 1], None, op0=eq
                )
                ps = pspool.tile([128, C], f32, tag="ps")
                nc.tensor.matmul(ps[:], m[:], ft[:], start=True, stop=True)
                ot = opool.tile([Q, C], f32, tag="ot")
                if COPY_ENG == "mix" and s % 2 == 0:
                    nc.scalar.copy(ot[:], ps[:Q, :])
                else:
                    nc.vector.tensor_copy(ot[:], ps[:Q, :])
                nc.gpsimd.indirect_dma_start(
                    out=out_d.ap(),
                    out_offset=bass.IndirectOffsetOnAxis(
                        ap=sidx_t[:Q, s : s + 1], axis=0
                    ),
                    in_=ot[:],
                    in_offset=None,
                )

    nc.compile()
    return nc


def _prepare_inputs(input_features, site_indices):
    feats = np.ascontiguousarray(np.asarray(input_features, dtype=np.float32))
    idx = np.asarray(site_indices).astype(np.int64)
    n = idx.shape[0]
    assert feats.shape == (n, C)

    core = idx >> 13
    local = idx & 8191
    blk = local >> 7
    pos = local & 127

    # Per (core, block): row count K and occupied-position count O.
    gblk = core * NBLK + blk
    Kmat = np.bincount(gblk, minlength=N_CORES * NBLK).reshape(N_CORES, NBLK)
    # occupied positions per (core, block)
    occ_key = np.unique(core * 8192 + local)
    Omat = np.bincount(occ_key >> 7, minlength=N_CORES * NBLK).reshape(N_CORES, NBLK)

    # Order blocks per core so slot capacities (cross-core maxima) stay tight.
    if SORT == "K":
        score = Kmat
    elif SORT == "O":
        score = Omat
    else:  # joint bytes: loads are 1 unit/row (bf16), stores 2 units/row
        score = Kmat + 2 * Omat
    order = np.argsort(-score, axis=1, kind="stable")  # [8, NBLK] -> block id

    Ksorted = np.take_along_axis(Kmat, order, axis=1)
    Osorted = np.take_along_axis(Omat, order, axis=1)
    KS = Ksorted.max(0)
    if ROUND > 1:
        KS = (KS + ROUND - 1) // ROUND * ROUND
    QS = Osorted.max(0)
    assert KS.max() <= 128, f"block row count {KS.max()} > 128 unsupported"
    CUM = np.concatenate([[0], np.cumsum(KS)]).astype(np.int64)
    TOTK = int(CUM[-1])

    fdt = ml_dtypes.bfloat16 if FDT == "bf16" else np.float32
    feats_pack = np.zeros((N_CORES, TOTK, C), dtype=fdt)
    rank_pack = np.full((N_CORES, 128, NBLK), -1.0, dtype=np.float32)
    sidx_pack = np.zeros((N_CORES, 128, NBLK), dtype=np.int32)

    # Sort rows by (core, block, pos) once; slice per (core, block).
    sort_key = (core << 13) | local
    row_order = np.argsort(sort_key, kind="stable")
    sorted_core = core[row_order]
    sorted_blk = blk[row_order]
    sorted_pos = pos[row_order]
    feats_sorted = feats[row_order].astype(fdt)

    # start offset of each (core, block) run within row_order
    run_starts = np.zeros(N_CORES * NBLK + 1, dtype=np.int64)
    np.cumsum(Kmat.ravel(), out=run_starts[1:])

    inv_order = np.empty_like(order)  # block id -> slot
    np.put_along_axis(inv_order, order, np.arange(NBLK)[None, :].repeat(8, 0), axis=1)

    for c in range(N_CORES):
        # pick a known-empty local row for store-index padding
        occ_local = np.unique(local[core == c])
        empty_mask = np.ones(POS_PER_CORE, dtype=bool)
        empty_mask[occ_local] = False
        trash = int(np.flatnonzero(empty_mask)[0])
        sidx_pack[c, :, :] = trash

        for b in range(NBLK):
            s = int(inv_order[c, b])
            r0, r1 = run_starts[c * NBLK + b], run_starts[c * NBLK + b + 1]
            k = int(r1 - r0)
            if k == 0:
                continue
            p_run = sorted_pos[r0:r1]
            uniq, ranks = np.unique(p_run, return_inverse=True)
            o = len(uniq)
            feats_pack[c, CUM[s] : CUM[s] + k, :] = feats_sorted[r0:r1]
            rank_pack[c, :k, s] = ranks.astype(np.float32)
            sidx_pack[c, :o, s] = (b * 128 + uniq).astype(np.int32)

    iota = np.tile(np.arange(128, dtype=np.float32), (128, 1))

    in_maps = []
    for c in range(N_CORES):
        in_maps.append(
            {
                "feats": feats_pack[c],
                "rank": rank_pack[c],
                "sidx": sidx_pack[c],
                "iota": iota,
            }
        )
    return in_maps, tuple(int(x) for x in KS), tuple(int(x) for x in QS)


def run(input_features, site_indices, trace: bool = False):
    in_maps, KS, QS = _prepare_inputs(input_features, site_indices)
    key = (KS, QS, FDT, COPY_ENG, FBUFS, OBUFS, MBUFS, PSBUFS)
    if key not in _PROGRAM_CACHE:
        _PROGRAM_CACHE[key] = _build_program(KS, QS)
    nc = _PROGRAM_CACHE[key]
    res = run_bass_kernel_spmd(nc, in_maps, list(range(N_CORES)), trace=trace)
    out = np.concatenate([res.results[c]["out"] for c in range(N_CORES)], axis=0)
    return out.reshape(B, L, C), res


def kernel(input_features, site_indices, batch_size, length):
    assert int(batch_size) == B and int(length) == L
    out, _ = run(input_features, site_indices, trace=False)
    return out


# revision 12
# speedup vs baseline: 2.5067x; 2.5067x over previous
"""Scatter-add of active-site feature rows into a dense (B, L, C) output,
distributed over 8 NeuronCores (data-parallel over the batch axis).

Core m owns flat output rows [m*8192, (m+1)*8192). Host-side, each core's
active rows are sorted by target row; rows are packed into chunks of <=128
(never splitting one target row's duplicate run across chunks, so each
occupied output row is produced by exactly one chunk). Each chunk is one
one-hot matmul

    acc[q, c] = sum_k 1{rank[k] == q} * feats[k, c]

accumulating duplicates in fp32 PSUM. The accumulator's 128 rows are then
scatter-stored with an indirect DMA: row q goes to the chunk's q-th distinct
target row; rows beyond the chunk's target count hold exact zeros (their
one-hot columns are empty) and are pointed at a known-empty "trash" row of
the same core, so every offset is valid. (The bounds_check/oob skip path is
NOT used: skipped offsets corrupt the descriptor stream on HW.) Untouched
output rows stay zero because run_bass_via_pjrt donates zero-initialized
output buffers (the native path pre-zeros them too).

SPMD uniformity: the chunk count NG is the max over cores, padded so the
store batches are all exactly `nb` columns (mixed-width indirect stores in
one program also derail HW descriptor generation); shorter cores pad with
empty chunks whose columns all point at the trash row.

Features travel as bf16, and the output tensor is bf16 on the wire (PSUM
accumulation is fp32; the host upcasts to fp32), halving both load and
store bytes for ~3e-3 relative error against the 2e-2 gate.
"""

import os

import numpy as np
import ml_dtypes

import concourse.bacc as bacc
import concourse.bass as bass
import concourse.mybir as mybir
import concourse.tile as tile
from concourse.bass_utils import run_bass_kernel_spmd

N_CORES = 8
B = 16
L = 4096
C = 512
POS_PER_CORE = B * L // N_CORES  # 8192

FDT = os.environ.get("K_FDT", "bf16")  # feature dtype on the wire
ODT = os.environ.get("K_ODT", "bf16")  # store dtype on the wire (f32|bf16)
# Single-column indirect stores only: with nb=1 the in_ AP's per-partition
# run equals one output row, so HW descriptor generation is unambiguous.
# Batched (nb>1) stores intermittently emit full-run descriptors on HW.
NBS = int(os.environ.get("K_NBS", "1"))  # target chunks per indirect store
NBL = int(os.environ.get("K_NBL", "4"))  # chunks batched per load DMA
FBUFS = int(os.environ.get("K_FBUFS", "4"))
OBUFS = int(os.environ.get("K_OBUFS", "3"))
MBUFS = int(os.environ.get("K_MBUFS", "6"))
PSBUFS = int(os.environ.get("K_PSBUFS", "8"))
COPY_ENG = os.environ.get("K_COPY", "mix")  # dve | mix

_PROGRAM_CACHE: dict = {}


def _batch_shape(NG: int):
    """Uniform store batching: nops ops of exactly nb columns each."""
    nops = -(-NG // NBS)
    nb = -(-NG // nops)
    return nops * nb, nb


def _build_program(NG: int, nb: int):
    f32 = mybir.dt.float32
    fdt = mybir.dt.bfloat16 if FDT == "bf16" else f32
    odt = mybir.dt.bfloat16 if ODT == "bf16" else f32
    i32 = mybir.dt.int32

    nc = bacc.Bacc(
        "TRN2",
        target_bir_lowering=False,
        debug=False,
        enable_asserts=False,
        num_devices=N_CORES,
    )
    # partition-major layout: row p holds chunk-row p of every chunk, so a
    # batch of NBL chunks loads as one DMA with NBL*C contiguous per partition
    feats_d = nc.dram_tensor("feats", [128, NG * C], fdt, kind="ExternalInput")
    rank_d = nc.dram_tensor("rank", [128, NG], f32, kind="ExternalInput")
    sidx_d = nc.dram_tensor("sidx", [128, NG], i32, kind="ExternalInput")
    iota_d = nc.dram_tensor("iota", [128, 128], f32, kind="ExternalInput")
    out_d = nc.dram_tensor("out", [POS_PER_CORE, C], odt, kind="ExternalOutput")

    eq = mybir.AluOpType.is_equal

    with tile.TileContext(nc) as tc:
        with (
            tc.tile_pool(name="const", bufs=1) as constp,
            tc.tile_pool(name="fpool", bufs=FBUFS) as fpool,
            tc.tile_pool(name="opool", bufs=OBUFS) as opool,
            tc.tile_pool(name="mpool", bufs=MBUFS) as mpool,
            tc.tile_pool(name="psum", bufs=PSBUFS, space="PSUM") as pspool,
        ):
            iota_t = constp.tile([128, 128], f32)
            nc.scalar.dma_start(iota_t[:], iota_d.ap())
            rank_t = constp.tile([128, NG], f32)
            nc.scalar.dma_start(rank_t[:], rank_d.ap())
            sidx_t = constp.tile([128, NG], i32)
            nc.scalar.dma_start(sidx_t[:], sidx_d.ap())

            ot = None
            ft = None
            for t in range(NG):
                jl = t % NBL
                if jl == 0:
                    nl = min(NBL, NG - t)  # chunks in this load batch
                    ft = fpool.tile([128, NBL * C], fdt, tag="ft")
                    nc.sync.dma_start(
                        ft[:, : nl * C],
                        feats_d.ap()[:, t * C : (t + nl) * C],
                    )
                m = mpool.tile([128, 128], fdt, tag="m")
                nc.vector.tensor_scalar(
                    m[:], iota_t[:], rank_t[:, t : t + 1], None, op0=eq
                )
                ps = pspool.tile([128, C], f32, tag="ps")
                nc.tensor.matmul(
                    ps[:], m[:], ft[:, jl * C : (jl + 1) * C], start=True, stop=True
                )
                j = t % nb
                if j == 0:
                    ot = opool.tile([128, nb * C], odt, tag="ot")
                if COPY_ENG == "mix" and t % 2 == 0:
                    nc.scalar.copy(ot[:, j * C : (j + 1) * C], ps[:])
                else:
                    nc.vector.tensor_copy(ot[:, j * C : (j + 1) * C], ps[:])
                if j == nb - 1:
                    t0 = t - j
                    nc.gpsimd.indirect_dma_start(
                        out=out_d.ap(),
                        out_offset=bass.IndirectOffsetOnAxis(
                            ap=sidx_t[:, t0 : t0 + nb], axis=0
                        ),
                        in_=ot[:],
                        in_offset=None,
                    )

    nc.compile()
    return nc


def _prepare_inputs(input_features, site_indices):
    feats = np.ascontiguousarray(np.asarray(input_features, dtype=np.float32))
    idx = np.asarray(site_indices).astype(np.int64)
    n = idx.shape[0]
    assert feats.shape == (n, C)

    core = idx >> 13
    local = idx & 8191

    order = np.argsort((core << 13) | local, kind="stable")
    score = core[order] * 8192 + local[order]  # sorted global target row

    # Per core: chunk the sorted rows into <=128-row chunks without splitting
    # one target row's duplicate run.
    chunk_of = np.empty(n, dtype=np.int64)  # chunk id per sorted row
    slot_of = np.empty(n, dtype=np.int64)  # partition slot within chunk
    rank_of = np.empty(n, dtype=np.int64)  # local rank within chunk
    chunk_targets = []  # per (core, chunk): np.array of local target rows
    chunk_counts = []  # per core: number of chunks
    core_starts = np.searchsorted(score, np.arange(N_CORES) * 8192)
    core_ends = np.append(core_starts[1:], n)

    for c in range(N_CORES):
        lo, hi = int(core_starts[c]), int(core_ends[c])
        svals = score[lo:hi] - c * 8192  # sorted local rows
        rs = np.flatnonzero(np.diff(svals, prepend=-1))  # start idx of each run
        run_len = np.diff(np.append(rs, hi - lo))
        targets = svals[rs]
        nchunk = 0
        fill = 0
        my_chunks = []
        cur_targets = []
        for r in range(len(rs)):
            ln = int(run_len[r])
            if fill + ln > 128:
                my_chunks.append(np.array(cur_targets, dtype=np.int64))
                cur_targets = []
                nchunk += 1
                fill = 0
            a = lo + int(rs[r])
            chunk_of[a : a + ln] = nchunk
            slot_of[a : a + ln] = fill + np.arange(ln)
            rank_of[a : a + ln] = len(cur_targets)
            cur_targets.append(int(targets[r]))
            fill += ln
        if fill:
            my_chunks.append(np.array(cur_targets, dtype=np.int64))
            nchunk += 1
        chunk_targets.append(my_chunks)
        chunk_counts.append(nchunk)

    NG, nb = _batch_shape(max(chunk_counts))
    fdt = ml_dtypes.bfloat16 if FDT == "bf16" else np.float32

    # partition-major feats: [128 partitions, NG chunks * C]
    feats_pack = np.zeros((N_CORES, 128, NG * C), dtype=fdt)
    rank_pack = np.full((N_CORES, 128, NG), -1.0, dtype=np.float32)
    sidx_pack = np.empty((N_CORES, 128, NG), dtype=np.int32)

    feats_sorted = feats[order].astype(fdt)
    col_of = chunk_of * C  # start column of each row's chunk
    for c in range(N_CORES):
        lo, hi = int(core_starts[c]), int(core_ends[c])
        cols = col_of[lo:hi, None] + np.arange(C)[None, :]
        feats_pack[c, slot_of[lo:hi, None], cols] = feats_sorted[lo:hi]
        rank_pack[c, slot_of[lo:hi], chunk_of[lo:hi]] = rank_of[lo:hi].astype(
            np.float32
        )
        # all-pad default: every index points at a known-empty local row, so
        # every descriptor is valid (zeros written there are a no-op)
        occ = np.unique(local[core == c])
        empty_mask = np.ones(POS_PER_CORE, dtype=bool)
        empty_mask[occ] = False
        trash = int(np.flatnonzero(empty_mask)[0])
        sidx_pack[c, :, :] = trash
        for t, tg in enumerate(chunk_targets[c]):
            sidx_pack[c, : len(tg), t] = tg.astype(np.int32)

    iota = np.tile(np.arange(128, dtype=np.float32), (128, 1))

    in_maps = []
    for c in range(N_CORES):
        in_maps.append(
            {
                "feats": feats_pack[c],
                "rank": rank_pack[c],
                "sidx": sidx_pack[c],
                "iota": iota,
            }
        )
    return in_maps, NG, nb


def run(input_features, site_indices, trace: bool = False):
    in_maps, NG, nb = _prepare_inputs(input_features, site_indices)
    key = (NG, nb, FDT, ODT, NBL, COPY_ENG, FBUFS, OBUFS, MBUFS, PSBUFS)
    if key not in _PROGRAM_CACHE:
        _PROGRAM_CACHE[key] = _build_program(NG, nb)
    nc = _PROGRAM_CACHE[key]
    res = run_bass_kernel_spmd(nc, in_maps, list(range(N_CORES)), trace=trace)
    out = np.concatenate(
        [np.asarray(res.results[c]["out"], dtype=np.float32) for c in range(N_CORES)],
        axis=0,
    )
    return out.reshape(B, L, C), res


def kernel(input_features, site_indices, batch_size, length):
    assert int(batch_size) == B and int(length) == L
    out, _ = run(input_features, site_indices, trace=False)
    return out


# revision 19
# speedup vs baseline: 4.6935x; 1.8724x over previous
"""Scatter-add of active-site feature rows into a dense (B, L, C) output,
distributed over 8 NeuronCores (data-parallel over the batch axis).

Core m owns flat output rows [m*8192, (m+1)*8192). Host-side, each core's
active rows are sorted by target row; rows are packed into chunks of <=128
(never splitting one target row's duplicate run across chunks, so each
occupied output row is produced by exactly one chunk). Each chunk is one
one-hot matmul

    acc[q, c] = sum_k 1{rank[k] == q} * feats[k, c]

accumulating duplicates in fp32 PSUM. The accumulator's 128 rows are then
scatter-stored with an indirect DMA: row q goes to the chunk's q-th distinct
target row; rows beyond the chunk's target count hold exact zeros (their
one-hot columns are empty) and are pointed at a known-empty "trash" row of
the same core, so every offset is valid. (The bounds_check/oob skip path is
NOT used: skipped offsets corrupt the descriptor stream on HW.) Untouched
output rows stay zero because run_bass_via_pjrt donates zero-initialized
output buffers (the native path pre-zeros them too).

SPMD uniformity: the chunk count NG is the max over cores, padded so the
store batches are all exactly `nb` columns (mixed-width indirect stores in
one program also derail HW descriptor generation); shorter cores pad with
empty chunks whose columns all point at the trash row.

Features travel as bf16, and the output tensor is bf16 on the wire (PSUM
accumulation is fp32; the host upcasts to fp32), halving both load and
store bytes for ~3e-3 relative error against the 2e-2 gate.
"""

import os

import numpy as np
import ml_dtypes

import concourse.bacc as bacc
import concourse.bass as bass
import concourse.mybir as mybir
import concourse.tile as tile
from concourse.bass_utils import run_bass_kernel_spmd

N_CORES = 8
B = 16
L = 4096
C = 512
POS_PER_CORE = B * L // N_CORES  # 8192

FDT = os.environ.get("K_FDT", "bf16")  # feature dtype on the wire
ODT = os.environ.get("K_ODT", "bf16")  # store dtype on the wire (f32|bf16)
# Single-column indirect stores only: with nb=1 the in_ AP's per-partition
# run equals one output row, so HW descriptor generation is unambiguous.
# Batched (nb>1) stores intermittently emit full-run descriptors on HW.
NBS = int(os.environ.get("K_NBS", "1"))  # target chunks per indirect store
NBL = int(os.environ.get("K_NBL", "4"))  # chunks batched per load DMA
FBUFS = int(os.environ.get("K_FBUFS", "4"))
OBUFS = int(os.environ.get("K_OBUFS", "3"))
MBUFS = int(os.environ.get("K_MBUFS", "6"))
PSBUFS = int(os.environ.get("K_PSBUFS", "8"))
COPY_ENG = os.environ.get("K_COPY", "mix")  # dve | mix

_PROGRAM_CACHE: dict = {}


def _batch_shape(NG: int):
    """Uniform store batching: nops ops of exactly nb columns each."""
    nops = -(-NG // NBS)
    nb = -(-NG // nops)
    return nops * nb, nb


def _build_program(NG: int, nb: int):
    f32 = mybir.dt.float32
    fdt = mybir.dt.bfloat16 if FDT == "bf16" else f32
    odt = mybir.dt.bfloat16 if ODT == "bf16" else f32
    i32 = mybir.dt.int32

    nc = bacc.Bacc(
        "TRN2",
        target_bir_lowering=False,
        debug=False,
        enable_asserts=False,
        num_devices=N_CORES,
    )
    # partition-major layout: row p holds chunk-row p of every chunk, so a
    # batch of NBL chunks loads as one DMA with NBL*C contiguous per partition
    feats_d = nc.dram_tensor("feats", [128, NG * C], fdt, kind="ExternalInput")
    rank_d = nc.dram_tensor("rank", [128, NG], f32, kind="ExternalInput")
    sidx_d = nc.dram_tensor("sidx", [128, NG], i32, kind="ExternalInput")
    iota_d = nc.dram_tensor("iota", [128, 128], f32, kind="ExternalInput")
    out_d = nc.dram_tensor("out", [POS_PER_CORE, C], odt, kind="ExternalOutput")

    eq = mybir.AluOpType.is_equal

    with tile.TileContext(nc) as tc:
        with (
            tc.tile_pool(name="const", bufs=1) as constp,
            tc.tile_pool(name="fpool", bufs=FBUFS) as fpool,
            tc.tile_pool(name="opool", bufs=OBUFS) as opool,
            tc.tile_pool(name="mpool", bufs=MBUFS) as mpool,
            tc.tile_pool(name="psum", bufs=PSBUFS, space="PSUM") as pspool,
        ):
            iota_t = constp.tile([128, 128], f32)
            nc.scalar.dma_start(iota_t[:], iota_d.ap())
            rank_t = constp.tile([128, NG], f32)
            nc.scalar.dma_start(rank_t[:], rank_d.ap())
            sidx_t = constp.tile([128, NG], i32)
            nc.scalar.dma_start(sidx_t[:], sidx_d.ap())

            ot = None
            ft = None
            for t in range(NG):
                jl = t % NBL
                if jl == 0:
                    nl = min(NBL, NG - t)  # chunks in this load batch
                    ft = fpool.tile([128, NBL * C], fdt, tag="ft")
                    nc.sync.dma_start(
                        ft[:, : nl * C],
                        feats_d.ap()[:, t * C : (t + nl) * C],
                    )
                m = mpool.tile([128, 128], fdt, tag="m")
                nc.vector.tensor_scalar(
                    m[:], iota_t[:], rank_t[:, t : t + 1], None, op0=eq
                )
                ps = pspool.tile([128, C], f32, tag="ps")
                nc.tensor.matmul(
                    ps[:], m[:], ft[:, jl * C : (jl + 1) * C], start=True, stop=True
                )
                j = t % nb
                if j == 0:
                    ot = opool.tile([128, nb * C], odt, tag="ot")
                if COPY_ENG == "mix" and t % 2 == 0:
                    nc.scalar.copy(ot[:, j * C : (j + 1) * C], ps[:])
                else:
                    nc.vector.tensor_copy(ot[:, j * C : (j + 1) * C], ps[:])
                if j == nb - 1:
                    t0 = t - j
                    if nb == 1:
                        # Disjoint dep-tracking ranges per store op: the
                        # tracker would otherwise serialize every indirect
                        # store on a write-after-write hazard over the whole
                        # out tensor (targets are disjoint by construction).
                        # The [1, C] shape keeps coef=C. Verified on HW at
                        # nb=1 only — batched stores with this fake AP
                        # scatter wrong, so nb>1 keeps the full AP and eats
                        # the WAW chain (few ops, mostly overlapped).
                        full = out_d.ap()
                        sl = full[0:1, :]
                        out_ap = bass.AP(
                            tensor=sl.tensor,
                            offset=0,
                            ap=sl.ap,
                            dep_tracking_offset=(t0 // nb) * C,
                        )
                    else:
                        out_ap = out_d.ap()
                    nc.gpsimd.indirect_dma_start(
                        out=out_ap,
                        out_offset=bass.IndirectOffsetOnAxis(
                            ap=sidx_t[:, t0 : t0 + nb], axis=0
                        ),
                        in_=ot[:],
                        in_offset=None,
                    )

    nc.compile()
    return nc


def _prepare_inputs(input_features, site_indices):
    feats = np.ascontiguousarray(np.asarray(input_features, dtype=np.float32))
    idx = np.asarray(site_indices).astype(np.int64)
    n = idx.shape[0]
    assert feats.shape == (n, C)

    core = idx >> 13
    local = idx & 8191

    order = np.argsort((core << 13) | local, kind="stable")
    score = core[order] * 8192 + local[order]  # sorted global target row

    # Per core: chunk the sorted rows into <=128-row chunks without splitting
    # one target row's duplicate run.
    chunk_of = np.empty(n, dtype=np.int64)  # chunk id per sorted row
    slot_of = np.empty(n, dtype=np.int64)  # partition slot within chunk
    rank_of = np.empty(n, dtype=np.int64)  # local rank within chunk
    chunk_targets = []  # per (core, chunk): np.array of local target rows
    chunk_counts = []  # per core: number of chunks
    core_starts = np.searchsorted(score, np.arange(N_CORES) * 8192)
    core_ends = np.append(core_starts[1:], n)

    for c in range(N_CORES):
        lo, hi = int(core_starts[c]), int(core_ends[c])
        svals = score[lo:hi] - c * 8192  # sorted local rows
        rs = np.flatnonzero(np.diff(svals, prepend=-1))  # start idx of each run
        run_len = np.diff(np.append(rs, hi - lo))
        targets = svals[rs]
        nchunk = 0
        fill = 0
        my_chunks = []
        cur_targets = []
        for r in range(len(rs)):
            ln = int(run_len[r])
            if fill + ln > 128:
                my_chunks.append(np.array(cur_targets, dtype=np.int64))
                cur_targets = []
                nchunk += 1
                fill = 0
            a = lo + int(rs[r])
            chunk_of[a : a + ln] = nchunk
            slot_of[a : a + ln] = fill + np.arange(ln)
            rank_of[a : a + ln] = len(cur_targets)
            cur_targets.append(int(targets[r]))
            fill += ln
        if fill:
            my_chunks.append(np.array(cur_targets, dtype=np.int64))
            nchunk += 1
        chunk_targets.append(my_chunks)
        chunk_counts.append(nchunk)

    NG, nb = _batch_shape(max(chunk_counts))
    fdt = ml_dtypes.bfloat16 if FDT == "bf16" else np.float32

    # partition-major feats: [128 partitions, NG chunks * C]
    feats_pack = np.zeros((N_CORES, 128, NG * C), dtype=fdt)
    rank_pack = np.full((N_CORES, 128, NG), -1.0, dtype=np.float32)
    sidx_pack = np.empty((N_CORES, 128, NG), dtype=np.int32)

    feats_sorted = feats[order].astype(fdt)
    col_of = chunk_of * C  # start column of each row's chunk
    for c in range(N_CORES):
        lo, hi = int(core_starts[c]), int(core_ends[c])
        cols = col_of[lo:hi, None] + np.arange(C)[None, :]
        feats_pack[c, slot_of[lo:hi, None], cols] = feats_sorted[lo:hi]
        rank_pack[c, slot_of[lo:hi], chunk_of[lo:hi]] = rank_of[lo:hi].astype(
            np.float32
        )
        # every pad index points at a distinct known-empty local row, so every
        # descriptor is valid (zeros written there are a no-op) and no two
        # descriptors in flight target the same address. Targets stay sorted
        # within a column: at nb=1 the SWDGE's consecutive-offset descriptor
        # merge is CORRECT (adjacent rows, adjacent data) and helps; only
        # multi-column ops mis-merge (which is why nb=1 is the default).
        occ = np.unique(local[core == c])
        empty_mask = np.ones(POS_PER_CORE, dtype=bool)
        empty_mask[occ] = False
        empties = np.flatnonzero(empty_mask)
        npad = 128 * NG - sum(len(tg) for tg in chunk_targets[c])
        assert npad <= len(empties), (npad, len(empties))
        pad_iter = iter(empties[:npad])
        for t in range(NG):
            tg = chunk_targets[c][t] if t < len(chunk_targets[c]) else []
            sidx_pack[c, : len(tg), t] = np.asarray(tg, dtype=np.int32)
            for q in range(len(tg), 128):
                sidx_pack[c, q, t] = next(pad_iter)

    iota = np.tile(np.arange(128, dtype=np.float32), (128, 1))

    in_maps = []
    for c in range(N_CORES):
        in_maps.append(
            {
                "feats": feats_pack[c],
                "rank": rank_pack[c],
                "sidx": sidx_pack[c],
                "iota": iota,
            }
        )
    return in_maps, NG, nb


def run(input_features, site_indices, trace: bool = False):
    in_maps, NG, nb = _prepare_inputs(input_features, site_indices)
    key = (NG, nb, FDT, ODT, NBL, COPY_ENG, FBUFS, OBUFS, MBUFS, PSBUFS)
    if key not in _PROGRAM_CACHE:
        _PROGRAM_CACHE[key] = _build_program(NG, nb)
    nc = _PROGRAM_CACHE[key]
    res = run_bass_kernel_spmd(nc, in_maps, list(range(N_CORES)), trace=trace)
    out = np.concatenate(
        [np.asarray(res.results[c]["out"], dtype=np.float32) for c in range(N_CORES)],
        axis=0,
    )
    return out.reshape(B, L, C), res


def kernel(input_features, site_indices, batch_size, length):
    assert int(batch_size) == B and int(length) == L
    out, _ = run(input_features, site_indices, trace=False)
    return out


# revision 21
# speedup vs baseline: 5.5891x; 1.1908x over previous
"""Scatter-add of active-site feature rows into a dense (B, L, C) output,
distributed over 8 NeuronCores (data-parallel over the batch axis).

Core m owns flat output rows [m*8192, (m+1)*8192). Host-side, each core's
active rows are sorted by target row; rows are packed into chunks of <=128
(never splitting one target row's duplicate run across chunks, so each
occupied output row is produced by exactly one chunk). Each chunk is one
one-hot matmul

    acc[q, c] = sum_k 1{rank[k] == q} * feats[k, c]

accumulating duplicates in fp32 PSUM. The accumulator's 128 rows are then
scatter-stored with an indirect DMA: row q goes to the chunk's q-th distinct
target row; rows beyond the chunk's target count hold exact zeros (their
one-hot columns are empty) and are pointed at a known-empty "trash" row of
the same core, so every offset is valid. (The bounds_check/oob skip path is
NOT used: skipped offsets corrupt the descriptor stream on HW.) Untouched
output rows stay zero because run_bass_via_pjrt donates zero-initialized
output buffers (the native path pre-zeros them too).

SPMD uniformity: the chunk count NG is the max over cores, padded so the
store batches are all exactly `nb` columns (mixed-width indirect stores in
one program also derail HW descriptor generation); shorter cores pad with
empty chunks whose columns all point at the trash row.

Features travel as bf16, and the output tensor is bf16 on the wire (PSUM
accumulation is fp32; the host upcasts to fp32), halving both load and
store bytes for ~3e-3 relative error against the 2e-2 gate.
"""

import os

import numpy as np
import ml_dtypes

import concourse.bacc as bacc
import concourse.bass as bass
import concourse.mybir as mybir
import concourse.tile as tile
from concourse.bass_utils import run_bass_kernel_spmd

N_CORES = 8
B = 16
L = 4096
C = 512
POS_PER_CORE = B * L // N_CORES  # 8192

FDT = os.environ.get("K_FDT", "bf16")  # feature dtype on the wire
ODT = os.environ.get("K_ODT", "bf16")  # store dtype on the wire (f32|bf16)
# Single-column indirect stores only: with nb=1 the in_ AP's per-partition
# run equals one output row, so HW descriptor generation is unambiguous.
# Batched (nb>1) stores intermittently emit full-run descriptors on HW.
NBS = int(os.environ.get("K_NBS", "1"))  # target chunks per indirect store
NBL = int(os.environ.get("K_NBL", "4"))  # chunks batched per load DMA
FBUFS = int(os.environ.get("K_FBUFS", "4"))
OBUFS = int(os.environ.get("K_OBUFS", "4"))
MBUFS = int(os.environ.get("K_MBUFS", "6"))
PSBUFS = int(os.environ.get("K_PSBUFS", "8"))
COPY_ENG = os.environ.get("K_COPY", "mix")  # dve | mix

_PROGRAM_CACHE: dict = {}


def _batch_shape(NG: int):
    """Uniform store batching: nops ops of exactly nb columns each."""
    nops = -(-NG // NBS)
    nb = -(-NG // nops)
    return nops * nb, nb


def _build_program(NG: int, nb: int):
    f32 = mybir.dt.float32
    fdt = mybir.dt.bfloat16 if FDT == "bf16" else f32
    odt = mybir.dt.bfloat16 if ODT == "bf16" else f32
    i32 = mybir.dt.int32

    nc = bacc.Bacc(
        "TRN2",
        target_bir_lowering=False,
        debug=False,
        enable_asserts=False,
        num_devices=N_CORES,
    )
    # partition-major layout: row p holds chunk-row p of every chunk, so a
    # batch of NBL chunks loads as one DMA with NBL*C contiguous per partition
    feats_d = nc.dram_tensor("feats", [128, NG * C], fdt, kind="ExternalInput")
    rank_d = nc.dram_tensor("rank", [128, NG], f32, kind="ExternalInput")
    sidx_d = nc.dram_tensor("sidx", [128, NG], i32, kind="ExternalInput")
    iota_d = nc.dram_tensor("iota", [128, 128], f32, kind="ExternalInput")
    out_d = nc.dram_tensor("out", [POS_PER_CORE, C], odt, kind="ExternalOutput")

    eq = mybir.AluOpType.is_equal

    with tile.TileContext(nc) as tc:
        with (
            tc.tile_pool(name="const", bufs=1) as constp,
            tc.tile_pool(name="fpool", bufs=FBUFS) as fpool,
            tc.tile_pool(name="opool", bufs=OBUFS) as opool,
            tc.tile_pool(name="mpool", bufs=MBUFS) as mpool,
            tc.tile_pool(name="psum", bufs=PSBUFS, space="PSUM") as pspool,
        ):
            # first feature batch issues ahead of the consts so the sync ring
            # starts streaming immediately (consts ride the scalar ring,
            # which first stalls ~1.3us on its activation-table load)
            ft0 = fpool.tile([128, NBL * C], fdt, tag="ft")
            nc.sync.dma_start(
                ft0[:, : min(NBL, NG) * C], feats_d.ap()[:, : min(NBL, NG) * C]
            )
            iota_t = constp.tile([128, 128], f32)
            nc.scalar.dma_start(iota_t[:], iota_d.ap())
            rank_t = constp.tile([128, NG], f32)
            nc.scalar.dma_start(rank_t[:], rank_d.ap())
            sidx_t = constp.tile([128, NG], i32)
            nc.scalar.dma_start(sidx_t[:], sidx_d.ap())

            ot = None
            ft = None
            for t in range(NG):
                jl = t % NBL
                if jl == 0:
                    if t == 0:
                        ft = ft0
                    else:
                        nl = min(NBL, NG - t)  # chunks in this load batch
                        ft = fpool.tile([128, NBL * C], fdt, tag="ft")
                        nc.sync.dma_start(
                            ft[:, : nl * C],
                            feats_d.ap()[:, t * C : (t + nl) * C],
                        )
                m = mpool.tile([128, 128], fdt, tag="m")
                nc.vector.tensor_scalar(
                    m[:], iota_t[:], rank_t[:, t : t + 1], None, op0=eq
                )
                ps = pspool.tile([128, C], f32, tag="ps")
                nc.tensor.matmul(
                    ps[:], m[:], ft[:, jl * C : (jl + 1) * C], start=True, stop=True
                )
                j = t % nb
                if j == 0:
                    ot = opool.tile([128, nb * C], odt, tag="ot")
                if COPY_ENG == "mix" and t % 2 == 0:
                    nc.scalar.copy(ot[:, j * C : (j + 1) * C], ps[:])
                else:
                    nc.vector.tensor_copy(ot[:, j * C : (j + 1) * C], ps[:])
                if j == nb - 1:
                    t0 = t - j
                    if nb == 1:
                        # Disjoint dep-tracking ranges per store op: the
                        # tracker would otherwise serialize every indirect
                        # store on a write-after-write hazard over the whole
                        # out tensor (targets are disjoint by construction).
                        # The [1, C] shape keeps coef=C. Verified on HW at
                        # nb=1 only — batched stores with this fake AP
                        # scatter wrong, so nb>1 keeps the full AP and eats
                        # the WAW chain (few ops, mostly overlapped).
                        full = out_d.ap()
                        sl = full[0:1, :]
                        out_ap = bass.AP(
                            tensor=sl.tensor,
                            offset=0,
                            ap=sl.ap,
                            dep_tracking_offset=(t0 // nb) * C,
                        )
                    else:
                        out_ap = out_d.ap()
                    nc.gpsimd.indirect_dma_start(
                        out=out_ap,
                        out_offset=bass.IndirectOffsetOnAxis(
                            ap=sidx_t[:, t0 : t0 + nb], axis=0
                        ),
                        in_=ot[:],
                        in_offset=None,
                    )

    nc.compile()
    return nc


def _prepare_inputs(input_features, site_indices):
    feats = np.ascontiguousarray(np.asarray(input_features, dtype=np.float32))
    idx = np.asarray(site_indices).astype(np.int64)
    n = idx.shape[0]
    assert feats.shape == (n, C)

    core = idx >> 13
    local = idx & 8191

    order = np.argsort((core << 13) | local, kind="stable")
    score = core[order] * 8192 + local[order]  # sorted global target row

    # Per core: chunk the sorted rows into <=128-row chunks without splitting
    # one target row's duplicate run.
    chunk_of = np.empty(n, dtype=np.int64)  # chunk id per sorted row
    slot_of = np.empty(n, dtype=np.int64)  # partition slot within chunk
    rank_of = np.empty(n, dtype=np.int64)  # local rank within chunk
    chunk_targets = []  # per (core, chunk): np.array of local target rows
    chunk_counts = []  # per core: number of chunks
    core_starts = np.searchsorted(score, np.arange(N_CORES) * 8192)
    core_ends = np.append(core_starts[1:], n)

    for c in range(N_CORES):
        lo, hi = int(core_starts[c]), int(core_ends[c])
        svals = score[lo:hi] - c * 8192  # sorted local rows
        rs = np.flatnonzero(np.diff(svals, prepend=-1))  # start idx of each run
        run_len = np.diff(np.append(rs, hi - lo))
        targets = svals[rs]
        nchunk = 0
        fill = 0
        my_chunks = []
        cur_targets = []
        for r in range(len(rs)):
            ln = int(run_len[r])
            if fill + ln > 128:
                my_chunks.append(np.array(cur_targets, dtype=np.int64))
                cur_targets = []
                nchunk += 1
                fill = 0
            a = lo + int(rs[r])
            chunk_of[a : a + ln] = nchunk
            slot_of[a : a + ln] = fill + np.arange(ln)
            rank_of[a : a + ln] = len(cur_targets)
            cur_targets.append(int(targets[r]))
            fill += ln
        if fill:
            my_chunks.append(np.array(cur_targets, dtype=np.int64))
            nchunk += 1
        chunk_targets.append(my_chunks)
        chunk_counts.append(nchunk)

    NG, nb = _batch_shape(max(chunk_counts))
    fdt = ml_dtypes.bfloat16 if FDT == "bf16" else np.float32

    # partition-major feats: [128 partitions, NG chunks * C]
    feats_pack = np.zeros((N_CORES, 128, NG * C), dtype=fdt)
    rank_pack = np.full((N_CORES, 128, NG), -1.0, dtype=np.float32)
    sidx_pack = np.empty((N_CORES, 128, NG), dtype=np.int32)

    feats_sorted = feats[order].astype(fdt)
    col_of = chunk_of * C  # start column of each row's chunk
    for c in range(N_CORES):
        lo, hi = int(core_starts[c]), int(core_ends[c])
        cols = col_of[lo:hi, None] + np.arange(C)[None, :]
        feats_pack[c, slot_of[lo:hi, None], cols] = feats_sorted[lo:hi]
        rank_pack[c, slot_of[lo:hi], chunk_of[lo:hi]] = rank_of[lo:hi].astype(
            np.float32
        )
        # every pad index points at a distinct known-empty local row, so every
        # descriptor is valid (zeros written there are a no-op) and no two
        # descriptors in flight target the same address. Targets stay sorted
        # within a column: at nb=1 the SWDGE's consecutive-offset descriptor
        # merge is CORRECT (adjacent rows, adjacent data) and helps; only
        # multi-column ops mis-merge (which is why nb=1 is the default).
        occ = np.unique(local[core == c])
        empty_mask = np.ones(POS_PER_CORE, dtype=bool)
        empty_mask[occ] = False
        empties = np.flatnonzero(empty_mask)
        npad = 128 * NG - sum(len(tg) for tg in chunk_targets[c])
        assert npad <= len(empties), (npad, len(empties))
        pad_iter = iter(empties[:npad])
        for t in range(NG):
            tg = chunk_targets[c][t] if t < len(chunk_targets[c]) else []
            sidx_pack[c, : len(tg), t] = np.asarray(tg, dtype=np.int32)
            for q in range(len(tg), 128):
                sidx_pack[c, q, t] = next(pad_iter)

    iota = np.tile(np.arange(128, dtype=np.float32), (128, 1))

    in_maps = []
    for c in range(N_CORES):
        in_maps.append(
            {
                "feats": feats_pack[c],
                "rank": rank_pack[c],
                "sidx": sidx_pack[c],
                "iota": iota,
            }
        )
    return in_maps, NG, nb


def run(input_features, site_indices, trace: bool = False):
    in_maps, NG, nb = _prepare_inputs(input_features, site_indices)
    key = (NG, nb, FDT, ODT, NBL, COPY_ENG, FBUFS, OBUFS, MBUFS, PSBUFS)
    if key not in _PROGRAM_CACHE:
        _PROGRAM_CACHE[key] = _build_program(NG, nb)
    nc = _PROGRAM_CACHE[key]
    res = run_bass_kernel_spmd(nc, in_maps, list(range(N_CORES)), trace=trace)
    out = np.concatenate(
        [np.asarray(res.results[c]["out"], dtype=np.float32) for c in range(N_CORES)],
        axis=0,
    )
    return out.reshape(B, L, C), res


def kernel(input_features, site_indices, batch_size, length):
    assert int(batch_size) == B and int(length) == L
    out, _ = run(input_features, site_indices, trace=False)
    return out


# revision 22
# speedup vs baseline: 5.7022x; 1.0202x over previous
"""Scatter-add of active-site feature rows into a dense (B, L, C) output,
distributed over 8 NeuronCores (data-parallel over the batch axis).

Core m owns flat output rows [m*8192, (m+1)*8192). Host-side, each core's
active rows are sorted by target row; rows are packed into chunks of <=128
(never splitting one target row's duplicate run across chunks, so each
occupied output row is produced by exactly one chunk). Each chunk is one
one-hot matmul

    acc[q, c] = sum_k 1{rank[k] == q} * feats[k, c]

accumulating duplicates in fp32 PSUM. The accumulator's 128 rows are then
scatter-stored with an indirect DMA: row q goes to the chunk's q-th distinct
target row; rows beyond the chunk's target count hold exact zeros (their
one-hot columns are empty) and are pointed at a known-empty "trash" row of
the same core, so every offset is valid. (The bounds_check/oob skip path is
NOT used: skipped offsets corrupt the descriptor stream on HW.) Untouched
output rows stay zero because run_bass_via_pjrt donates zero-initialized
output buffers (the native path pre-zeros them too).

SPMD uniformity: the chunk count NG is the max over cores, padded so the
store batches are all exactly `nb` columns (mixed-width indirect stores in
one program also derail HW descriptor generation); shorter cores pad with
empty chunks whose columns all point at the trash row.

Features travel as bf16, and the output tensor is bf16 on the wire (PSUM
accumulation is fp32; the host upcasts to fp32), halving both load and
store bytes for ~3e-3 relative error against the 2e-2 gate.
"""

import os

import numpy as np
import ml_dtypes

import concourse.bacc as bacc
import concourse.bass as bass
import concourse.mybir as mybir
import concourse.tile as tile
from concourse.bass_utils import run_bass_kernel_spmd

N_CORES = 8
B = 16
L = 4096
C = 512
POS_PER_CORE = B * L // N_CORES  # 8192

FDT = os.environ.get("K_FDT", "bf16")  # feature dtype on the wire
ODT = os.environ.get("K_ODT", "bf16")  # store dtype on the wire (f32|bf16)
# Single-column indirect stores only: with nb=1 the in_ AP's per-partition
# run equals one output row, so HW descriptor generation is unambiguous.
# Batched (nb>1) stores intermittently emit full-run descriptors on HW.
NBS = int(os.environ.get("K_NBS", "1"))  # target chunks per indirect store
NBL = int(os.environ.get("K_NBL", "4"))  # chunks batched per load DMA
FBUFS = int(os.environ.get("K_FBUFS", "4"))
OBUFS = int(os.environ.get("K_OBUFS", "4"))
MBUFS = int(os.environ.get("K_MBUFS", "6"))
PSBUFS = int(os.environ.get("K_PSBUFS", "8"))
COPY_ENG = os.environ.get("K_COPY", "mix")  # dve | mix

_PROGRAM_CACHE: dict = {}


def _batch_shape(NG: int):
    """Uniform store batching: nops ops of exactly nb columns each."""
    nops = -(-NG // NBS)
    nb = -(-NG // nops)
    return nops * nb, nb


def _build_program(NG: int, nb: int):
    f32 = mybir.dt.float32
    fdt = mybir.dt.bfloat16 if FDT == "bf16" else f32
    odt = mybir.dt.bfloat16 if ODT == "bf16" else f32
    i32 = mybir.dt.int32

    nc = bacc.Bacc(
        "TRN2",
        target_bir_lowering=False,
        debug=False,
        enable_asserts=False,
        num_devices=N_CORES,
    )
    # partition-major layout: row p holds chunk-row p of every chunk, so a
    # batch of NBL chunks loads as one DMA with NBL*C contiguous per partition
    feats_d = nc.dram_tensor("feats", [128, NG * C], fdt, kind="ExternalInput")
    rank_d = nc.dram_tensor("rank", [128, NG], f32, kind="ExternalInput")
    sidx_d = nc.dram_tensor("sidx", [128, NG], i32, kind="ExternalInput")
    iota_d = nc.dram_tensor("iota", [128, 128], f32, kind="ExternalInput")
    out_d = nc.dram_tensor("out", [POS_PER_CORE, C], odt, kind="ExternalOutput")

    eq = mybir.AluOpType.is_equal

    with tile.TileContext(nc) as tc:
        with (
            tc.tile_pool(name="const", bufs=1) as constp,
            tc.tile_pool(name="fpool", bufs=FBUFS) as fpool,
            tc.tile_pool(name="opool", bufs=OBUFS) as opool,
            tc.tile_pool(name="mpool", bufs=MBUFS) as mpool,
            tc.tile_pool(name="psum", bufs=PSBUFS, space="PSUM") as pspool,
        ):
            # first feature batch issues ahead of the consts so the sync ring
            # starts streaming immediately (consts ride the scalar ring,
            # which first stalls ~1.3us on its activation-table load)
            ft0 = fpool.tile([128, NBL * C], fdt, tag="ft")
            nc.sync.dma_start(
                ft0[:, : min(NBL, NG) * C], feats_d.ap()[:, : min(NBL, NG) * C]
            )
            # consts on the sync ring too: the scalar ring stalls ~1.3us on
            # its activation-table load before its first DMA can issue
            iota_t = constp.tile([128, 128], f32)
            nc.sync.dma_start(iota_t[:], iota_d.ap())
            rank_t = constp.tile([128, NG], f32)
            nc.sync.dma_start(rank_t[:], rank_d.ap())
            sidx_t = constp.tile([128, NG], i32)
            nc.scalar.dma_start(sidx_t[:], sidx_d.ap())

            ot = None
            ft = None
            for t in range(NG):
                jl = t % NBL
                if jl == 0:
                    if t == 0:
                        ft = ft0
                    else:
                        nl = min(NBL, NG - t)  # chunks in this load batch
                        ft = fpool.tile([128, NBL * C], fdt, tag="ft")
                        nc.sync.dma_start(
                            ft[:, : nl * C],
                            feats_d.ap()[:, t * C : (t + nl) * C],
                        )
                m = mpool.tile([128, 128], fdt, tag="m")
                nc.vector.tensor_scalar(
                    m[:], iota_t[:], rank_t[:, t : t + 1], None, op0=eq
                )
                ps = pspool.tile([128, C], f32, tag="ps")
                nc.tensor.matmul(
                    ps[:], m[:], ft[:, jl * C : (jl + 1) * C], start=True, stop=True
                )
                j = t % nb
                if j == 0:
                    ot = opool.tile([128, nb * C], odt, tag="ot")
                if COPY_ENG == "mix" and t % 2 == 0:
                    nc.scalar.copy(ot[:, j * C : (j + 1) * C], ps[:])
                else:
                    nc.vector.tensor_copy(ot[:, j * C : (j + 1) * C], ps[:])
                if j == nb - 1:
                    t0 = t - j
                    if nb == 1:
                        # Disjoint dep-tracking ranges per store op: the
                        # tracker would otherwise serialize every indirect
                        # store on a write-after-write hazard over the whole
                        # out tensor (targets are disjoint by construction).
                        # The [1, C] shape keeps coef=C. Verified on HW at
                        # nb=1 only — batched stores with this fake AP
                        # scatter wrong, so nb>1 keeps the full AP and eats
                        # the WAW chain (few ops, mostly overlapped).
                        full = out_d.ap()
                        sl = full[0:1, :]
                        out_ap = bass.AP(
                            tensor=sl.tensor,
                            offset=0,
                            ap=sl.ap,
                            dep_tracking_offset=(t0 // nb) * C,
                        )
                    else:
                        out_ap = out_d.ap()
                    nc.gpsimd.indirect_dma_start(
                        out=out_ap,
                        out_offset=bass.IndirectOffsetOnAxis(
                            ap=sidx_t[:, t0 : t0 + nb], axis=0
                        ),
                        in_=ot[:],
                        in_offset=None,
                    )

    nc.compile()
    return nc


def _prepare_inputs(input_features, site_indices):
    feats = np.ascontiguousarray(np.asarray(input_features, dtype=np.float32))
    idx = np.asarray(site_indices).astype(np.int64)
    n = idx.shape[0]
    assert feats.shape == (n, C)

    core = idx >> 13
    local = idx & 8191

    order = np.argsort((core << 13) | local, kind="stable")
    score = core[order] * 8192 + local[order]  # sorted global target row

    # Per core: chunk the sorted rows into <=128-row chunks without splitting
    # one target row's duplicate run.
    chunk_of = np.empty(n, dtype=np.int64)  # chunk id per sorted row
    slot_of = np.empty(n, dtype=np.int64)  # partition slot within chunk
    rank_of = np.empty(n, dtype=np.int64)  # local rank within chunk
    chunk_targets = []  # per (core, chunk): np.array of local target rows
    chunk_counts = []  # per core: number of chunks
    core_starts = np.searchsorted(score, np.arange(N_CORES) * 8192)
    core_ends = np.append(core_starts[1:], n)

    for c in range(N_CORES):
        lo, hi = int(core_starts[c]), int(core_ends[c])
        svals = score[lo:hi] - c * 8192  # sorted local rows
        rs = np.flatnonzero(np.diff(svals, prepend=-1))  # start idx of each run
        run_len = np.diff(np.append(rs, hi - lo))
        targets = svals[rs]
        nchunk = 0
        fill = 0
        my_chunks = []
        cur_targets = []
        for r in range(len(rs)):
            ln = int(run_len[r])
            if fill + ln > 128:
                my_chunks.append(np.array(cur_targets, dtype=np.int64))
                cur_targets = []
                nchunk += 1
                fill = 0
            a = lo + int(rs[r])
            chunk_of[a : a + ln] = nchunk
            slot_of[a : a + ln] = fill + np.arange(ln)
            rank_of[a : a + ln] = len(cur_targets)
            cur_targets.append(int(targets[r]))
            fill += ln
        if fill:
            my_chunks.append(np.array(cur_targets, dtype=np.int64))
            nchunk += 1
        chunk_targets.append(my_chunks)
        chunk_counts.append(nchunk)

    NG, nb = _batch_shape(max(chunk_counts))
    fdt = ml_dtypes.bfloat16 if FDT == "bf16" else np.float32

    # partition-major feats: [128 partitions, NG chunks * C]
    feats_pack = np.zeros((N_CORES, 128, NG * C), dtype=fdt)
    rank_pack = np.full((N_CORES, 128, NG), -1.0, dtype=np.float32)
    sidx_pack = np.empty((N_CORES, 128, NG), dtype=np.int32)

    feats_sorted = feats[order].astype(fdt)
    col_of = chunk_of * C  # start column of each row's chunk
    for c in range(N_CORES):
        lo, hi = int(core_starts[c]), int(core_ends[c])
        cols = col_of[lo:hi, None] + np.arange(C)[None, :]
        feats_pack[c, slot_of[lo:hi, None], cols] = feats_sorted[lo:hi]
        rank_pack[c, slot_of[lo:hi], chunk_of[lo:hi]] = rank_of[lo:hi].astype(
            np.float32
        )
        # every pad index points at a distinct known-empty local row, so every
        # descriptor is valid (zeros written there are a no-op) and no two
        # descriptors in flight target the same address. Targets stay sorted
        # within a column: at nb=1 the SWDGE's consecutive-offset descriptor
        # merge is CORRECT (adjacent rows, adjacent data) and helps; only
        # multi-column ops mis-merge (which is why nb=1 is the default).
        occ = np.unique(local[core == c])
        empty_mask = np.ones(POS_PER_CORE, dtype=bool)
        empty_mask[occ] = False
        empties = np.flatnonzero(empty_mask)
        npad = 128 * NG - sum(len(tg) for tg in chunk_targets[c])
        assert npad <= len(empties), (npad, len(empties))
        pad_iter = iter(empties[:npad])
        for t in range(NG):
            tg = chunk_targets[c][t] if t < len(chunk_targets[c]) else []
            sidx_pack[c, : len(tg), t] = np.asarray(tg, dtype=np.int32)
            for q in range(len(tg), 128):
                sidx_pack[c, q, t] = next(pad_iter)

    iota = np.tile(np.arange(128, dtype=np.float32), (128, 1))

    in_maps = []
    for c in range(N_CORES):
        in_maps.append(
            {
                "feats": feats_pack[c],
                "rank": rank_pack[c],
                "sidx": sidx_pack[c],
                "iota": iota,
            }
        )
    return in_maps, NG, nb


def run(input_features, site_indices, trace: bool = False):
    in_maps, NG, nb = _prepare_inputs(input_features, site_indices)
    key = (NG, nb, FDT, ODT, NBL, COPY_ENG, FBUFS, OBUFS, MBUFS, PSBUFS)
    if key not in _PROGRAM_CACHE:
        _PROGRAM_CACHE[key] = _build_program(NG, nb)
    nc = _PROGRAM_CACHE[key]
    res = run_bass_kernel_spmd(nc, in_maps, list(range(N_CORES)), trace=trace)
    out = np.concatenate(
        [np.asarray(res.results[c]["out"], dtype=np.float32) for c in range(N_CORES)],
        axis=0,
    )
    return out.reshape(B, L, C), res


def kernel(input_features, site_indices, batch_size, length):
    assert int(batch_size) == B and int(length) == L
    out, _ = run(input_features, site_indices, trace=False)
    return out
